# revision 6
# baseline (speedup 1.0000x reference)
"""BottAttention Trainium2 kernel.

Reference computation (per batch b):
    qkv = x @ W_qkv                       # [N, 3*H*D]
    q,k,v per head h (D=64)
    S = q @ k.T * D**-0.5                 # [N, N]
    P = softmax(S, axis=-1) + reg[h]      # post-softmax learned bias
    o = P @ v                             # [N, D]
    out = concat_h(o) @ W_proj + b_proj   # [N, C]

Sharding across 8 cores: 2 batch-groups x 4 head-groups.  Core c handles
batches 4*(c//4)..+4 and heads 4*(c%4)..+4 and produces the partial
projection output for its heads (transposed, [C, N] per batch).  The host
sums the 4 head-group partials per batch, transposes back, and adds b_proj.

On-device layout trick: the host supplies x and reg pre-transposed (xT
[C,N] and regT [N,N] per head).  Then every matmul consumes its operands
in natural PE orientation and no on-chip transposes are needed:
    qT,kT  = (W_qk tiles).T @ xT          # [wcol, tok]   (lhsT = W)
    v      = (xT tiles).T @ W_v           # [tok, vcol]   (lhsT = xT)
    ST     = kT.T @ qT                    # [j, i] scores transposed
    expST  = exp(ST * scale)              # ACT, psum -> sbuf bf16
    avT    = [v | 1].T @ expST            # [65, i]: rows 0-63 = (attn@v).T
                                          #          row 64    = softmax rowsum
    rgT    = v.T @ regT                   # [64, i] = (reg@v).T
    attnT  = avT * bcast(1/rowsum) + rgT  # [hd, i]
    outT   = W_proj_rows.T @ attnT        # [C, i] partial
"""

import os
import numpy as np
import ml_dtypes
from contextlib import ExitStack

import concourse.bass as bass
import concourse.bacc as bacc
import concourse.tile as tile
from concourse import mybir
from concourse.bass_utils import run_bass_kernel_spmd

FP32 = mybir.dt.float32
FR = mybir.dt.float32r
BF16 = mybir.dt.bfloat16

# Problem dims (hardcoded per contest contract)
B, H, N, C, D = 8, 16, 1024, 1024, 64
SCALE = D ** -0.5

# Per-core shard dims
NB = 4          # batches per core
NH = 4          # heads per core
KC = C // 128   # contraction tiles for dim C
TT = N // 128   # token tiles
NBLK = 512      # matmul moving-dim block
IB = N // NBLK  # i blocks

N_CORES = 8


def build_program():
    nc = bacc.Bacc("TRN2", debug=False, enable_asserts=False, num_devices=1)

    xT = nc.dram_tensor("xT", [NB, C, N], FR, kind="ExternalInput").ap()
    wqk = nc.dram_tensor("wqk", [C, 4 * 128], FR, kind="ExternalInput").ap()
    wv = nc.dram_tensor("wv", [C, NH * D], FR, kind="ExternalInput").ap()
    regT = nc.dram_tensor("regT", [NH, N, N], BF16, kind="ExternalInput").ap()
    wp = nc.dram_tensor("wp", [NH * D, C], FR, kind="ExternalInput").ap()
    outp = nc.dram_tensor("outp", [NB, C, N], FP32, kind="ExternalOutput").ap()

    xT_r = xT.rearrange("b (kc p) n -> b kc p n", p=128)
    wqk_r = wqk.rearrange("(kc p) m -> kc p m", p=128)
    wv_r = wv.rearrange("(kc p) m -> kc p m", p=128)
    regT_r = regT.rearrange("h (tt p) n -> h tt p n", p=128)
    wp_r = wp.rearrange("(kt p) c -> kt p c", p=128)
    outp_r = outp.rearrange("b (ct p) n -> b ct p n", p=128)

    EXPF = mybir.ActivationFunctionType.Exp

    with (
        nc.allow_low_precision(reason="fp32r feeds the PE; bf16 attn path is intentional"),
        tile.TileContext(nc) as tc,
        ExitStack() as top,
    ):
        persist = top.enter_context(tc.tile_pool(name="persist", bufs=1))

        ones_f = persist.tile([1, 64], FP32, tag="ones_f")
        nc.vector.memset(ones_f[:], 1.0)
        ones_t = persist.tile([1, 64], FR, tag="ones")
        nc.vector.tensor_copy(ones_t[:], ones_f[:])

        # qk_sb[b][m]: m=0,1 -> qT head pairs (0,1),(2,3); m=2,3 -> kT pairs
        qk_sb = [
            [persist.tile([128, N], FR, tag=f"qk{b}_{m}", name=f"qk{b}_{m}") for m in range(4)]
            for b in range(NB)
        ]
        # v_sb[b][t]: [128, NH*65] bf16; per head slot: 64 v cols + ones col
        v_sb = [
            [persist.tile([128, NH * 65], BF16, tag=f"v{b}_{t}", name=f"v{b}_{t}") for t in range(TT)]
            for b in range(NB)
        ]
        wp_sb = [persist.tile([128, C], FR, tag=f"wp{k}", name=f"wp{k}") for k in range(2)]
        for k in range(2):
            nc.sync.dma_start(wp_sb[k][:], wp_r[k])

        # ---------------- Phase A: QKV projection ----------------
        with (
            tc.tile_pool(name="wq", bufs=1) as wpool,
            tc.tile_pool(name="xt", bufs=9) as xpool,
            tc.tile_pool(name="psA", bufs=2, space="PSUM") as psA,
            tc.tile_pool(name="psV", bufs=2, space="PSUM") as psV,
        ):
            wqk_sb = [wpool.tile([128, 4 * 128], FR, tag=f"wqk{k}", name=f"wqk{k}") for k in range(KC)]
            wv_sb = [wpool.tile([128, NH * D], FR, tag=f"wv{k}", name=f"wv{k}") for k in range(KC)]
            for k in range(KC):
                nc.sync.dma_start(wqk_sb[k][:], wqk_r[k])
                nc.sync.dma_start(wv_sb[k][:], wv_r[k])

            for b in range(NB):
                xts = []
                for k in range(KC):
                    t = xpool.tile([128, N], FR, tag="xt")
                    nc.sync.dma_start(t[:], xT_r[b, k])
                    xts.append(t)
                # qT / kT tiles: out[wcol, tok]
                for m in range(4):
                    for tb in range(IB):
                        ps = psA.tile([128, NBLK], FP32, tag="ps")
                        for k in range(KC):
                            nc.tensor.matmul(
                                ps[:],
                                lhsT=wqk_sb[k][:, m * 128:(m + 1) * 128],
                                rhs=xts[k][:, tb * NBLK:(tb + 1) * NBLK],
                                start=(k == 0),
                                stop=(k == KC - 1),
                            )
                        nc.vector.tensor_copy(
                            qk_sb[b][m][:, tb * NBLK:(tb + 1) * NBLK], ps[:]
                        )
                # v tiles: out[tok, vcol], written into per-head slots + ones col
                for tt in range(TT):
                    ps = psV.tile([128, NH * D], FP32, tag="psv")
                    for k in range(KC):
                        nc.tensor.matmul(
                            ps[:],
                            lhsT=xts[k][:, tt * 128:(tt + 1) * 128],
                            rhs=wv_sb[k][:],
                            start=(k == 0),
                            stop=(k == KC - 1),
                        )
                    vd = v_sb[b][tt][:].rearrange("p (h s) -> p h s", s=65)
                    nc.vector.tensor_copy(
                        vd[:, :, 0:64], ps[:].rearrange("p (h s) -> p h s", s=64)
                    )
                    nc.vector.memset(vd[:, :, 64:65], 1.0)

        # ---------------- Phase B: attention ----------------
        att_pool = top.enter_context(tc.tile_pool(name="att", bufs=1))
        att_sb = [
            [att_pool.tile([128, N], FR, tag=f"at{b}_{k}", name=f"at{b}_{k}") for k in range(2)]
            for b in range(NB)
        ]

        with (
            tc.tile_pool(name="reg", bufs=18) as rpool,
            tc.tile_pool(name="est", bufs=20) as epool,
            tc.tile_pool(name="sm", bufs=3) as small,
            tc.tile_pool(name="psS", bufs=3, space="PSUM") as psS,
            tc.tile_pool(name="psAv", bufs=2, space="PSUM") as psAv,
            tc.tile_pool(name="psRg", bufs=2, space="PSUM") as psRg,
            tc.tile_pool(name="psRb", bufs=1, space="PSUM") as psRb,
        ):
            for ib in range(IB):
                for hp in range(NH // 2):
                    # reg half-tiles for this head pair / i-block
                    rgt = {}
                    for h2 in range(2):
                        h = hp * 2 + h2
                        for j in range(TT):
                            t = rpool.tile([128, NBLK], BF16, tag="reg")
                            nc.sync.dma_start(
                                t[:], regT_r[h, j, :, ib * NBLK:(ib + 1) * NBLK]
                            )
                            rgt[(h2, j)] = t
                    for b in range(NB):
                        qt = qk_sb[b][hp]
                        kt = qk_sb[b][2 + hp]
                        # scores (transposed) + exp, both heads interleaved so
                        # the PE can overlap the K=64 matmuls via row groups
                        est = {}
                        for j in range(TT):
                            for h2 in range(2):
                                po = h2 * 64
                                ps = psS.tile([128, NBLK], FP32, tag="st")
                                nc.tensor.matmul(
                                    ps[:],
                                    lhsT=kt[po:po + 64, j * 128:(j + 1) * 128],
                                    rhs=qt[po:po + 64, ib * NBLK:(ib + 1) * NBLK],
                                    start=True,
                                    stop=True,
                                )
                                e = epool.tile([128, NBLK], BF16, tag="est")
                                nc.scalar.activation(e[:], ps[:], EXPF, scale=SCALE)
                                est[(h2, j)] = e
                        for h2 in range(2):
                            h = hp * 2 + h2
                            po = h2 * 64
                            vslots = [
                                v_sb[b][t][:].rearrange("p (hh s) -> p hh s", s=65)[:, h, :]
                                for t in range(TT)
                            ]
                            # unnormalized (attn @ v).T with rowsum in row 64
                            av = psAv.tile([128, NBLK], FP32, tag="av")
                            for j in range(TT):
                                nc.tensor.matmul(
                                    av[0:65, :],
                                    lhsT=vslots[j],
                                    rhs=est[(h2, j)][:],
                                    start=(j == 0),
                                    stop=(j == TT - 1),
                                )
                            # (reg @ v).T
                            rg = psRg.tile([128, NBLK], FP32, tag="rg")
                            for j in range(TT):
                                nc.tensor.matmul(
                                    rg[0:64, :],
                                    lhsT=vslots[j][:, 0:64],
                                    rhs=rgt[(h2, j)][:],
                                    start=(j == 0),
                                    stop=(j == TT - 1),
                                )
                            # normalize + add reg term
                            rcp = small.tile([1, NBLK], FR, tag="rcp")
                            nc.vector.reciprocal(rcp[:], av[64:65, :])
                            rb = psRb.tile([64, NBLK], FP32, tag="rb")
                            nc.tensor.matmul(
                                rb[:],
                                lhsT=ones_t[:],
                                rhs=rcp[:],
                                start=True,
                                stop=True,
                            )
                            rbs = small.tile([64, NBLK], FP32, tag="rbs")
                            nc.scalar.copy(rbs[:], rb[:])
                            tmp = small.tile([64, NBLK], FP32, tag="tmp")
                            nc.vector.tensor_mul(tmp[:], av[0:64, :], rbs[:])
                            nc.vector.tensor_add(
                                att_sb[b][hp][po:po + 64, ib * NBLK:(ib + 1) * NBLK],
                                tmp[:],
                                rg[0:64, :],
                            )

        # ---------------- Phase C: output projection ----------------
        with (
            tc.tile_pool(name="psC", bufs=4, space="PSUM") as psC,
            tc.tile_pool(name="outs", bufs=4) as outs_pool,
        ):
            for b in range(NB):
                for ct in range(KC):
                    for ibb in range(IB):
                        ps = psC.tile([128, NBLK], FP32, tag="pc")
                        for k in range(2):
                            nc.tensor.matmul(
                                ps[:],
                                lhsT=wp_sb[k][:, ct * 128:(ct + 1) * 128],
                                rhs=att_sb[b][k][:, ibb * NBLK:(ibb + 1) * NBLK],
                                start=(k == 0),
                                stop=(k == 1),
                            )
                        ot = outs_pool.tile([128, NBLK], FP32, tag="ot")
                        nc.scalar.copy(ot[:], ps[:])
                        nc.sync.dma_start(
                            outp_r[b, ct, :, ibb * NBLK:(ibb + 1) * NBLK], ot[:]
                        )

    nc.compile()
    return nc


_NC = None


def _get_program():
    global _NC
    if _NC is None:
        _NC = build_program()
    return _NC


def make_in_maps(x, W_qkv, reg):
    """Host-side sharding: per-core input dicts."""
    x = np.asarray(x, dtype=np.float32)
    W_qkv = np.asarray(W_qkv, dtype=np.float32)
    reg = np.asarray(reg, dtype=np.float32)
    in_maps = []
    for c in range(N_CORES):
        bg, hg = divmod(c, 4)
        xT_c = np.ascontiguousarray(
            x[bg * NB:(bg + 1) * NB].transpose(0, 2, 1)
        )
        q_cols = W_qkv[:, hg * NH * D:(hg + 1) * NH * D]
        k_cols = W_qkv[:, H * D + hg * NH * D:H * D + (hg + 1) * NH * D]
        wqk_c = np.ascontiguousarray(np.concatenate([q_cols, k_cols], axis=1))
        wv_c = np.ascontiguousarray(
            W_qkv[:, 2 * H * D + hg * NH * D:2 * H * D + (hg + 1) * NH * D]
        )
        regT_c = np.ascontiguousarray(
            reg[0, hg * NH:(hg + 1) * NH].transpose(0, 2, 1)
        ).astype(ml_dtypes.bfloat16)
        in_maps.append({"xT": xT_c, "wqk": wqk_c, "wv": wv_c, "regT": regT_c})
    return in_maps


def assemble_output(results, W_proj_unused, b_proj):
    b_proj = np.asarray(b_proj, dtype=np.float32)
    out = np.empty((B, N, C), dtype=np.float32)
    for b in range(B):
        bg, bl = divmod(b, NB)
        acc = results[bg * 4 + 0]["outp"][bl].copy()
        for hg in range(1, 4):
            acc += results[bg * 4 + hg]["outp"][bl]
        out[b] = acc.T + b_proj
    return out


def kernel(x, W_qkv, reg, W_proj, b_proj, trace=None):
    if trace is None:
        trace = bool(int(os.environ.get("KERNEL_TRACE", "0")))
    nc = _get_program()
    in_maps = make_in_maps(x, W_qkv, reg)
    W_proj = np.asarray(W_proj, dtype=np.float32)
    for c in range(N_CORES):
        hg = c % 4
        in_maps[c]["wp"] = np.ascontiguousarray(
            W_proj[hg * NH * D:(hg + 1) * NH * D, :]
        )
    res = run_bass_kernel_spmd(nc, in_maps, core_ids=list(range(N_CORES)), trace=trace)
    kernel.last_results = res
    return assemble_output(res.results, W_proj, b_proj)


# revision 10
# speedup vs baseline: 1.6563x; 1.6563x over previous
"""BottAttention Trainium2 kernel.

Reference computation (per batch b):
    qkv = x @ W_qkv                       # [N, 3*H*D]
    q,k,v per head h (D=64)
    S = q @ k.T * D**-0.5                 # [N, N]
    P = softmax(S, axis=-1) + reg[h]      # post-softmax learned bias
    o = P @ v                             # [N, D]
    out = concat_h(o) @ W_proj + b_proj   # [N, C]

Sharding across 8 cores: 2 batch-groups x 4 head-groups.  Core c handles
batches 4*(c//4)..+4 and heads 4*(c%4)..+4 and produces the partial
projection output for its heads (transposed, [C, N] per batch).  The host
sums the 4 head-group partials per batch, transposes back, and adds b_proj.

On-device layout trick: the host supplies x and reg pre-transposed (xT
[C,N] and regT [N,N] per head).  Then every matmul consumes its operands
in natural PE orientation and no on-chip transposes are needed:
    qT,kT  = (W_qk tiles).T @ xT          # [wcol, tok]   (lhsT = W)
    v      = (xT tiles).T @ W_v           # [tok, vcol]   (lhsT = xT)
    ST     = kT.T @ qT                    # [j, i] scores transposed
    expST  = exp(ST * scale)              # ACT, psum -> sbuf bf16
    avT    = [v | 1].T @ expST            # [65, i]: rows 0-63 = (attn@v).T
                                          #          row 64    = softmax rowsum
    rgT    = v.T @ regT                   # [64, i] = (reg@v).T (2 batches/matmul)
    attnT  = avT * bcast(1/rowsum) + rgT  # [hd, i]
    outT   = W_proj_rows.T @ attnT        # [C, i] partial

Perf notes: matmul dtypes are float32r (full PE rate at N>=512) for
QKV/scores/proj and bf16 for the attention-weight matmuls; scores for the
two heads of a pair sit at partition offsets 0/64 so their K=64 matmuls
run concurrently in disjoint PE row groups; softmax reciprocal uses the
fast approx + a GpSimd partition_broadcast; the projection for each
i-half is emitted as soon as that half of attnT is complete.
"""

import os
import numpy as np
import ml_dtypes
from contextlib import ExitStack

import concourse.bass as bass
import concourse.bacc as bacc
import concourse.tile as tile
from concourse import mybir
from concourse.bass_utils import run_bass_kernel_spmd

FP32 = mybir.dt.float32
FR = mybir.dt.float32r
BF16 = mybir.dt.bfloat16

# Problem dims (hardcoded per contest contract)
B, H, N, C, D = 8, 16, 1024, 1024, 64
SCALE = D ** -0.5

# Per-core shard dims
NB = 4          # batches per core
NH = 4          # heads per core
KC = C // 128   # contraction tiles for dim C
TT = N // 128   # token tiles
NBLK = 512      # matmul moving-dim block
IB = N // NBLK  # i blocks

N_CORES = 8


def build_program():
    nc = bacc.Bacc("TRN2", debug=False, enable_asserts=False, num_devices=1)

    xT = nc.dram_tensor("xT", [NB, C, N], FR, kind="ExternalInput").ap()
    wqk = nc.dram_tensor("wqk", [C, 4 * 128], FR, kind="ExternalInput").ap()
    wv = nc.dram_tensor("wv", [C, NH * D], FR, kind="ExternalInput").ap()
    regT = nc.dram_tensor("regT", [NH, N, N], BF16, kind="ExternalInput").ap()
    wp = nc.dram_tensor("wp", [NH * D, C], FR, kind="ExternalInput").ap()
    outp = nc.dram_tensor("outp", [NB, C, N], FP32, kind="ExternalOutput").ap()

    xT_r = xT.rearrange("b (kc p) n -> b kc p n", p=128)
    wqk_r = wqk.rearrange("(kc p) m -> kc p m", p=128)
    wv_r = wv.rearrange("(kc p) m -> kc p m", p=128)
    regT_r = regT.rearrange("h (tt p) n -> h p tt n", p=128)  # [NH,128,TT,N]
    wp_r = wp.rearrange("(kt p) c -> kt p c", p=128)
    outp_r = outp.rearrange("b (ct p) n -> b ct p n", p=128)

    EXPF = mybir.ActivationFunctionType.Exp

    with (
        nc.allow_low_precision(reason="fp32r feeds the PE; bf16 attn path is intentional"),
        tile.TileContext(nc) as tc,
        ExitStack() as top,
    ):
        persist = top.enter_context(tc.tile_pool(name="persist", bufs=1))

        # qk_sb[b][m]: m=0,1 -> qT head pairs (0,1),(2,3); m=2,3 -> kT pairs
        qk_sb = [
            [persist.tile([128, N], FR, tag=f"qk{b}_{m}", name=f"qk{b}_{m}")
             for m in range(4)]
            for b in range(NB)
        ]
        # v_sb[t]: [128, NB, NH, 65] bf16; per (batch, head) slot: 64 v cols
        # + a ones column (yields the softmax rowsum for free in the AV matmul)
        v_sb = [
            persist.tile([128, NB, NH, 65], BF16, tag=f"v{t}", name=f"v{t}")
            for t in range(TT)
        ]
        wp_sb = [persist.tile([128, C], FR, tag=f"wp{k}", name=f"wp{k}") for k in range(2)]
        for k in range(2):
            nc.sync.dma_start(wp_sb[k][:], wp_r[k])

        # ---------------- Phase A: QKV projection ----------------
        with (
            tc.tile_pool(name="wq", bufs=1) as wpool,
            tc.tile_pool(name="xt", bufs=17) as xpool,
            tc.tile_pool(name="psA", bufs=2, space="PSUM") as psA,
            tc.tile_pool(name="psV", bufs=2, space="PSUM") as psV,
        ):
            wqk_sb = [wpool.tile([128, 4 * 128], FR, tag=f"wqk{k}", name=f"wqk{k}")
                      for k in range(KC)]
            wv_sb = [wpool.tile([128, NH * D], FR, tag=f"wv{k}", name=f"wv{k}")
                     for k in range(KC)]
            for k in range(KC):
                nc.sync.dma_start(wqk_sb[k][:], wqk_r[k])
                nc.sync.dma_start(wv_sb[k][:], wv_r[k])

            for b in range(NB):
                xts = []
                for k in range(KC):
                    t = xpool.tile([128, N], FR, tag="xt")
                    nc.sync.dma_start(t[:], xT_r[b, k])
                    xts.append(t)
                # qT / kT tiles: out[wcol, tok]
                for m in range(4):
                    for tb in range(IB):
                        ps = psA.tile([128, NBLK], FP32, tag="ps")
                        for k in range(KC):
                            nc.tensor.matmul(
                                ps[:],
                                lhsT=wqk_sb[k][:, m * 128:(m + 1) * 128],
                                rhs=xts[k][:, tb * NBLK:(tb + 1) * NBLK],
                                start=(k == 0),
                                stop=(k == KC - 1),
                            )
                        nc.vector.tensor_copy(
                            qk_sb[b][m][:, tb * NBLK:(tb + 1) * NBLK], ps[:]
                        )
                # v tiles: out[tok, vcol], written into per-(batch,head) slots
                for tt in range(TT):
                    ps = psV.tile([128, NH * D], FP32, tag="psv")
                    for k in range(KC):
                        nc.tensor.matmul(
                            ps[:],
                            lhsT=xts[k][:, tt * 128:(tt + 1) * 128],
                            rhs=wv_sb[k][:],
                            start=(k == 0),
                            stop=(k == KC - 1),
                        )
                    vd = v_sb[tt][:]
                    nc.vector.tensor_copy(
                        vd[:, b, :, 0:64], ps[:].rearrange("p (h s) -> p h s", s=64)
                    )
                    nc.vector.memset(vd[:, b, :, 64:65], 1.0)

        # ---------------- Phases B+C: attention + projection ----------------
        att_pool = top.enter_context(tc.tile_pool(name="att", bufs=1))
        att_sb = [
            [att_pool.tile([128, N], FR, tag=f"at{b}_{k}", name=f"at{b}_{k}")
             for k in range(2)]
            for b in range(NB)
        ]

        with (
            tc.tile_pool(name="reg", bufs=3) as rpool,
            tc.tile_pool(name="est", bufs=10) as epool,
            tc.tile_pool(name="sm", bufs=2) as small,
            tc.tile_pool(name="outs", bufs=2) as outs_pool,
            tc.tile_pool(name="psS", bufs=2, space="PSUM") as psS,
            tc.tile_pool(name="psAv", bufs=2, space="PSUM") as psAv,
            tc.tile_pool(name="psRg", bufs=1, space="PSUM") as psRg,
            tc.tile_pool(name="psC", bufs=1, space="PSUM") as psC,
        ):
            for ib in range(IB):
                for hp in range(NH // 2):
                    # reg tiles (bf16, [128, TT, 512]) for this head pair / i-block
                    regt = {}
                    for h2 in range(2):
                        h = hp * 2 + h2
                        t = rpool.tile([128, TT, NBLK], BF16, tag="reg")
                        nc.sync.dma_start(
                            t[:], regT_r[h, :, :, ib * NBLK:(ib + 1) * NBLK]
                        )
                        regt[h2] = t
                    for b in range(NB):
                        if True:
                            qt = qk_sb[b][hp]
                            kt = qk_sb[b][2 + hp]
                            # transposed scores for both heads of the pair:
                            # partition offsets 0/64 -> concurrent PE row groups
                            ests = []
                            for j in range(TT):
                                ps = psS.tile([128, 2, NBLK], FP32, tag="st")
                                for h2 in range(2):
                                    po = h2 * 64
                                    nc.tensor.matmul(
                                        ps[:, h2, :],
                                        lhsT=kt[po:po + 64, j * 128:(j + 1) * 128],
                                        rhs=qt[po:po + 64, ib * NBLK:(ib + 1) * NBLK],
                                        start=True,
                                        stop=True,
                                    )
                                e = epool.tile([128, 2, NBLK], BF16, tag="est")
                                nc.scalar.activation(e[:], ps[:], EXPF, scale=SCALE)
                                ests.append(e)
                            for h2 in range(2):
                                h = hp * 2 + h2
                                av = psAv.tile([128, NBLK], FP32, tag="av")
                                for j in range(TT):
                                    nc.tensor.matmul(
                                        av[0:65, :],
                                        lhsT=v_sb[j][:, b, h, :],
                                        rhs=ests[j][:, h2, :],
                                        start=(j == 0),
                                        stop=(j == TT - 1),
                                    )
                                # custom-DVE recip mis-reads PSUM; stage via SBUF
                                rsum = small.tile([1, NBLK], FP32, tag="rsum")
                                nc.vector.tensor_copy(rsum[:], av[64:65, :])
                                rcp = small.tile([1, NBLK], FP32, tag="rcp")
                                nc.vector.reciprocal_approx_fast(rcp[:], rsum[:])
                                rbc = small.tile([64, NBLK], FP32, tag="rbc")
                                nc.gpsimd.partition_broadcast(rbc[:], rcp[:], channels=64)
                                tmp = small.tile([64, NBLK], FP32, tag="tmp", bufs=5)
                                nc.vector.tensor_mul(tmp[:], av[0:64, :], rbc[:])
                                # (reg @ v).T for this (batch, head)
                                rgp = psRg.tile([128, NBLK], FP32, tag="rg")
                                for j in range(TT):
                                    nc.tensor.matmul(
                                        rgp[0:64, :],
                                        lhsT=v_sb[j][:, b, h, 0:64],
                                        rhs=regt[h2][:, j, :],
                                        start=(j == 0),
                                        stop=(j == TT - 1),
                                    )
                                nc.vector.tensor_add(
                                    att_sb[b][hp][h2 * 64:(h2 + 1) * 64,
                                                  ib * NBLK:(ib + 1) * NBLK],
                                    tmp[:],
                                    rgp[0:64, :],
                                )
                # projection for this i-half (overlaps the next i-half's phase B)
                for b in range(NB):
                    for ct in range(KC):
                        ps = psC.tile([128, NBLK], FP32, tag="pc")
                        for k in range(2):
                            nc.tensor.matmul(
                                ps[:],
                                lhsT=wp_sb[k][:, ct * 128:(ct + 1) * 128],
                                rhs=att_sb[b][k][:, ib * NBLK:(ib + 1) * NBLK],
                                start=(k == 0),
                                stop=(k == 1),
                            )
                        ot = outs_pool.tile([128, NBLK], FP32, tag="ot")
                        nc.scalar.copy(ot[:], ps[:])
                        nc.sync.dma_start(
                            outp_r[b, ct, :, ib * NBLK:(ib + 1) * NBLK], ot[:]
                        )

    nc.compile()
    return nc


_NC = None


def _get_program():
    global _NC
    if _NC is None:
        _NC = build_program()
    return _NC


def make_in_maps(x, W_qkv, reg):
    """Host-side sharding: per-core input dicts."""
    x = np.asarray(x, dtype=np.float32)
    W_qkv = np.asarray(W_qkv, dtype=np.float32)
    reg = np.asarray(reg, dtype=np.float32)
    in_maps = []
    for c in range(N_CORES):
        bg, hg = divmod(c, 4)
        xT_c = np.ascontiguousarray(
            x[bg * NB:(bg + 1) * NB].transpose(0, 2, 1)
        )
        q_cols = W_qkv[:, hg * NH * D:(hg + 1) * NH * D]
        k_cols = W_qkv[:, H * D + hg * NH * D:H * D + (hg + 1) * NH * D]
        wqk_c = np.ascontiguousarray(np.concatenate([q_cols, k_cols], axis=1))
        wv_c = np.ascontiguousarray(
            W_qkv[:, 2 * H * D + hg * NH * D:2 * H * D + (hg + 1) * NH * D]
        )
        regT_c = np.ascontiguousarray(
            reg[0, hg * NH:(hg + 1) * NH].transpose(0, 2, 1)
        ).astype(ml_dtypes.bfloat16)
        in_maps.append({"xT": xT_c, "wqk": wqk_c, "wv": wv_c, "regT": regT_c})
    return in_maps


def assemble_output(results, b_proj):
    b_proj = np.asarray(b_proj, dtype=np.float32)
    out = np.empty((B, N, C), dtype=np.float32)
    for b in range(B):
        bg, bl = divmod(b, NB)
        acc = results[bg * 4 + 0]["outp"][bl].copy()
        for hg in range(1, 4):
            acc += results[bg * 4 + hg]["outp"][bl]
        out[b] = acc.T + b_proj
    return out


def kernel(x, W_qkv, reg, W_proj, b_proj, trace=None):
    if trace is None:
        trace = bool(int(os.environ.get("KERNEL_TRACE", "0")))
    nc = _get_program()
    in_maps = make_in_maps(x, W_qkv, reg)
    W_proj = np.asarray(W_proj, dtype=np.float32)
    for c in range(N_CORES):
        hg = c % 4
        in_maps[c]["wp"] = np.ascontiguousarray(
            W_proj[hg * NH * D:(hg + 1) * NH * D, :]
        )
    res = run_bass_kernel_spmd(nc, in_maps, core_ids=list(range(N_CORES)), trace=trace)
    kernel.last_results = res
    return assemble_output(res.results, b_proj)
